# revision 28
# baseline (speedup 1.0000x reference)
"""Trainium2 Bass kernel: fused attention block (QKV proj -> MHA -> out proj).

Reference (per batch item b, NUM_HEADS=12, Dh=64):
    qkv = x @ W_qkv; q,k,v per head
    attn = softmax(q @ k^T / 8) @ v
    out  = concat_heads(attn) @ W_proj + b_proj

Sharding: data-parallel over batch across 8 NeuronCores (128 batch items
per core), weights replicated. One SPMD Bass program, per-core inputs.

Per-core plan (128 batches, groups of G=8 batches = 392 tokens). All
matmul operands bf16 (fp32 accumulate in PSUM). x is pre-transposed +
cast on host to feature-major xT[768, 6272] per core, so no on-device
transposes. Software-pipelined one group deep: attention+proj of group
g-1 is emitted alongside QKV of group g so the PE never idles.

  B. q,k GEMM feature-major: psum[128co, T] = Wqkv_tile.T @ xT.
     q co-tiles stored naturally [128, T+2]; k co-tiles DVE-scattered
     into block-diagonal kbd[g%2]: per (j, b) a [128, 128] block with
     k_h(2j) rows 0:64 cols 0:49 and k_h(2j+1) rows 64:128 cols 64:113
     (zeros elsewhere, memset once).
  C. v GEMM token-major over 98-token tiles -> scratch; 4 aligned
     SBUF->SBUF DMAs per tile scatter into vbd[g%2]: per (j, b) a
     [128, 128] block with v_h(2j) rows 0:49 cols 0:64 and v_h(2j+1)
     rows 64:113 cols 64:128.
  D. attention per (head-pair j, batch): both heads in one matmul chain:
       sT2 = kbd_jb.T @ q_pair        [128, 50]  scores, heads stacked
       eT2 = exp(sT2/8)               ACT, [128, 400] per j
       r2  = onesbd.T @ eT2           [2, 400]   row sums per head
       po  = vbd_jb.T @ eT2           [128, 50]  unnorm out^T, both heads
     r rows gathered (ACT copy + SBUF-shift DMA) into rgrp[12, T];
     one batched reciprocal per group; per j: broadcast matmul
     (sel2 selector, f32r) -> psum bc[128, T]; unT[j] *= bc (DVE).
  E. proj GEMM token-major: psum[98tok, 384] = unT_slice.T @ Wproj,
     bias accumulated via ones-row matmul -> copy -> DMA out.
"""
import sys

sys.path.insert(0, "/opt/trn_rl_repo")

import numpy as np

NUM_CORES = 8
B_CORE = 128          # batch items per core
SEQ = 49              # tokens per batch item
C = 768               # channels
H = 12                # heads
G = 8                 # batch items per group
T = SEQ * G           # 392 tokens per group (even)
TP = T + 2            # padded q tile width
TOK = B_CORE * SEQ    # 6272 tokens per core
N_GROUPS = B_CORE // G
BDW = G * 128         # kbd/vbd block row width per j (padded 128/batch)

_CACHE = {}


def _consts():
    import ml_dtypes
    bf16 = ml_dtypes.bfloat16
    # sel2: broadcast head sums over Dh partitions (bf16 matmul);
    # contraction padded to 128 rows (sub-128-partition matmuls run slow)
    sel2 = np.zeros((128, 6 * 128), dtype=bf16)
    for j in range(6):
        sel2[2 * j, 128 * j:128 * j + 64] = 1.0
        sel2[2 * j + 1, 128 * j + 64:128 * (j + 1)] = 1.0
    # onesbd6: per head-pair j a [128, 32] stationary with ones at col 2j
    # (head-even keys, eT rows 0:49) and col 2j+1 (head-odd, rows 64:113);
    # all 6 r matmuls accumulate into one [32, 400] psum tile
    onesbd = np.zeros((128, 6 * 128), dtype=bf16)
    for j in range(6):
        onesbd[0:49, 128 * j + 2 * j] = 1.0
        onesbd[64:113, 128 * j + 2 * j + 1] = 1.0
    return {"sel2": sel2, "onesbd": onesbd}


def _build():
    import concourse.bacc as bacc
    import concourse.mybir as mybir
    import concourse.tile as tile

    F32 = mybir.dt.float32
    F32R = mybir.dt.float32r
    BF16 = mybir.dt.bfloat16
    EXP = mybir.ActivationFunctionType.Exp

    nc = bacc.Bacc("TRN2", target_bir_lowering=False)

    d_xt = nc.declare_dram_parameter("xt", [C, TOK + 2], BF16, isOutput=False)
    d_wqkv = nc.declare_dram_parameter("wqkv", [C, 3 * C], BF16, isOutput=False)
    d_wproj = nc.declare_dram_parameter("wproj", [C, C], BF16, isOutput=False)
    d_biast = nc.declare_dram_parameter("biast", [128, 6], BF16,
                                        isOutput=False)
    d_sel2 = nc.declare_dram_parameter("sel2", [128, 6 * 128], BF16,
                                       isOutput=False)
    d_onesbd = nc.declare_dram_parameter("onesbd", [128, 6 * 128], BF16,
                                         isOutput=False)
    d_out = nc.declare_dram_parameter("out", [C, TOK], F32, isOutput=True)

    # 98-token tiles within a group (4 per group, batch-pair aligned)
    tts = [(o, 98) for o in range(0, T, 98)]

    with tile.TileContext(nc) as tc, \
         nc.allow_low_precision(reason="bf16 operands, fp32 accumulate"):
        with tc.tile_pool(name="wres", bufs=1) as wres, \
             tc.tile_pool(name="xT", bufs=2) as p_xT, \
             tc.tile_pool(name="qk", bufs=2) as p_qk, \
             tc.tile_pool(name="vscr", bufs=2) as p_vscr, \
             tc.tile_pool(name="eT", bufs=2) as p_eT, \
             tc.tile_pool(name="rr", bufs=2) as p_rr, \
             tc.tile_pool(name="unT", bufs=3) as p_unT, \
             tc.tile_pool(name="osb", bufs=2) as p_osb, \
             tc.tile_pool(name="psA", bufs=2, space="PSUM") as psA, \
             tc.tile_pool(name="psB", bufs=2, space="PSUM") as psB, \
             tc.tile_pool(name="psS", bufs=1, space="PSUM") as psS, \
             tc.tile_pool(name="psO", bufs=2, space="PSUM") as psO, \
             tc.tile_pool(name="psR", bufs=1, space="PSUM") as psR:

            state = {}

            def issue_x_dma(g, split=False):
                xT = p_xT.tile([128, 6 * TP], BF16, tag="xT", name="xT")
                xv = xT[:].rearrange("p (ci t) -> p ci t", t=TP)
                src_ap = d_xt[:].rearrange("(ci p) t -> p ci t", ci=6)
                if split:
                    for ci in range(6):
                        nc.sync.dma_start(
                            xv[:, ci:ci + 1, :],
                            src_ap[:, ci:ci + 1, g * T:g * T + TP])
                else:
                    nc.sync.dma_start(xv, src_ap[:, :, g * T:g * T + TP])
                state[("xv", g)] = xv

            # ---- resident weights / constants ----
            # first co-slice of each w_qkv tile first, then group 0+1's x,
            # then the weight remainders: the PE's first LDW chain only
            # needs [*, 0:128] of each tile
            w_qkv = []
            for ci in range(6):
                t = wres.tile([128, 3 * C], BF16, tag=f"wqkv{ci}")
                nc.sync.dma_start(t[:, 0:128],
                                  d_wqkv[128 * ci:128 * (ci + 1), 0:128])
                w_qkv.append(t)
            issue_x_dma(0, split=True)
            for ci in range(6):
                nc.sync.dma_start(
                    w_qkv[ci][:, 128:640],
                    d_wqkv[128 * ci:128 * (ci + 1), 128:640])
            issue_x_dma(1, split=True)
            for ci in range(6):
                nc.sync.dma_start(
                    w_qkv[ci][:, 640:],
                    d_wqkv[128 * ci:128 * (ci + 1), 640:])
            w_proj = []
            for ci in range(6):
                t = wres.tile([128, C], BF16, tag=f"wproj{ci}")
                nc.sync.dma_start(t[:], d_wproj[128 * ci:128 * (ci + 1), :])
                w_proj.append(t)
            sel2 = wres.tile([128, 6 * 128], BF16, tag="sel2")
            nc.sync.dma_start(sel2[:], d_sel2[:])
            onesbd = wres.tile([128, 6 * 128], BF16, tag="onesbd")
            nc.sync.dma_start(onesbd[:], d_onesbd[:])
            biast = wres.tile([128, 6], BF16, tag="biast")
            nc.sync.dma_start(biast[:], d_biast[:])
            # double-buffered block-diagonal k/v, zeroed once
            kbds, vbds = [], []
            for s in range(2):
                kb = wres.tile([128, 6 * BDW], BF16, tag=f"kbd{s}")
                nc.vector.memset(kb[:], 0.0)
                kbds.append(kb)
                vb = wres.tile([128, 6 * BDW], BF16, tag=f"vbd{s}")
                nc.vector.memset(vb[:], 0.0)
                vbds.append(vb)

            def qk_cotile(g, j):
                """One qkv co-tile: 6 MMs; q copy or k block-diag scatter."""
                kbd = kbds[g % 2]
                xv = state[("xv", g)]
                pq = psA.tile([128, TP], F32, tag="psA", name="psa")
                for ci in range(6):
                    nc.tensor.matmul(
                        pq[:, :TP],
                        w_qkv[ci][:, 128 * j:128 * (j + 1)],
                        xv[:, ci, :],
                        start=(ci == 0), stop=(ci == 5))
                if j < 6:
                    t = p_qk.tile([128, TP], BF16, tag=f"q{j}", name=f"q{j}")
                    if j % 2 == 0:
                        nc.scalar.copy(t[:], pq[:])
                    else:
                        nc.vector.tensor_copy(t[:], pq[:])
                    state[("q", g, j)] = t
                else:
                    jj = j - 6
                    kv = kbd[:, jj * BDW:(jj + 1) * BDW].rearrange(
                        "p (b two t) -> p b two t", two=2, t=64)
                    nc.scalar.copy(
                        kv[0:64, :, 0, 0:49],
                        pq[0:64, :T].rearrange("p (b t) -> p b t", t=49))
                    nc.scalar.copy(
                        kv[64:128, :, 1, 0:49],
                        pq[64:128, :T].rearrange("p (b t) -> p b t", t=49))

            def v_tile(g, tti):
                """One 98-token v tile: 12 MMs + 2 copies + 4 scatter DMAs."""
                vbd = vbds[g % 2]
                xv = state[("xv", g)]
                v5 = vbd[:].rearrange("p (j b two c) -> p j b two c",
                                      b=G, two=2, c=64)
                to, tk = tts[tti]
                scr = p_vscr.tile([98, C], BF16, tag="vscr", name="vscr")
                for half in range(2):
                    pv = psB.tile([98, 384], F32, tag="psB", name="psb")
                    for ci in range(6):
                        nc.tensor.matmul(
                            pv[:, :],
                            xv[:, ci, to:to + tk],
                            w_qkv[ci][:, 1536 + 384 * half:
                                      1536 + 384 * (half + 1)],
                            start=(ci == 0), stop=(ci == 5))
                    nc.vector.tensor_copy(
                        scr[:, 384 * half:384 * (half + 1)], pv[:, :])
                sv = scr[:].rearrange("p (j two c) -> p j two c",
                                      two=2, c=64)
                for bl in range(2):
                    b = 2 * tti + bl
                    for two in range(2):
                        nc.sync.dma_start(
                            v5[64 * two:64 * two + 49, :, b, two, :],
                            sv[49 * bl:49 * bl + 49, :, two, :])

            def attn_sc(g, j):
                """Scores + exp for head-pair j of group g."""
                kbd = kbds[g % 2]
                if j == 0:
                    state[("unT", g)] = [
                        p_unT.tile([128, T], BF16, tag=f"unT{ci}",
                                   name=f"unT{ci}") for ci in range(6)]
                    state[("rall", g)] = psR.tile([128, 50 * G], F32,
                                                  tag="rall", name="rall")
                qj = state.pop(("q", g, j))
                ps = psS.tile([128, 50 * G], F32, tag="psS", name="ps")
                for b in range(G):
                    nc.tensor.matmul(
                        ps[:, 50 * b:50 * b + 50],
                        kbd[:, j * BDW + 128 * b:j * BDW + 128 * (b + 1)],
                        qj[:, 49 * b:49 * b + 50],
                        start=True, stop=True)
                eT = p_eT.tile([128, 50 * G], BF16, tag="eT", name="eT")
                nc.scalar.activation(eT[:], ps[:], EXP, scale=0.125)
                state[("eT", g, j)] = eT

            def attn_fin(g, j):
                """Row sums + po + unT extraction for head-pair j."""
                vbd = vbds[g % 2]
                eT = state.pop(("eT", g, j))
                unT = state[("unT", g)]
                rall = state[("rall", g)]
                nc.tensor.matmul(rall[:], onesbd[:, 128 * j:128 * (j + 1)],
                                 eT[:], start=(j == 0), stop=(j == 5))
                po = psO.tile([128, 50 * G], F32, tag="psO", name="po")
                for b in range(G):
                    nc.tensor.matmul(
                        po[:, 50 * b:50 * b + 50],
                        vbd[:, j * BDW + 128 * b:j * BDW + 128 * (b + 1)],
                        eT[:, 50 * b:50 * b + 50],
                        start=True, stop=True)
                nc.vector.tensor_copy(
                    unT[j][:, :].rearrange("p (b c) -> p b c", c=49),
                    po[:, :].rearrange("p (b c) -> p b c", c=50)[:, :, 0:49])
                if j == 5:
                    rall = state.pop(("rall", g))
                    rgrp = p_rr.tile([H, T], F32, tag="rgrp", name="rgrp")
                    nc.scalar.copy(
                        rgrp[:].rearrange("p (b c) -> p b c", c=49),
                        rall[0:H, :].rearrange(
                            "p (b c) -> p b c", c=50)[:, :, 0:49])
                    state[("rgrp", g)] = rgrp

            def recip(g):
                rgrp = state.pop(("rgrp", g))
                rr32 = p_rr.tile([H, T], F32, tag="rr32", name="rr32")
                nc.vector.reciprocal_approx_fast(rr32[:], rgrp[:])
                rr = p_rr.tile([128, T], BF16, tag="rr", name="rr")
                nc.vector.memset(rr[:], 0.0)
                nc.vector.tensor_copy(rr[0:H, :], rr32[:])
                state[("rr", g)] = rr

            def bcmul(g, j):
                """Broadcast 1/r over Dh partitions; normalize unT[j]."""
                rr = state[("rr", g)]
                unT = state[("unT", g)]
                pbc = psA.tile([128, TP], F32, tag="psA", name="psa")
                nc.tensor.matmul(pbc[:, :T], sel2[:, 128 * j:128 * (j + 1)],
                                 rr[:], start=True, stop=True)
                nc.vector.tensor_mul(out=unT[j][:, :], in0=unT[j][:, :],
                                     in1=pbc[:, :T])
                if j == 5:
                    del state[("rr", g)]

            def proj_co(g, co):
                """One out co-tile: 6 MMs + bias add + DMA out."""
                r0 = g * T
                unT = state[("unT", g)]
                pp = psA.tile([128, TP], F32, tag="psA", name="psa")
                for ci in range(6):
                    nc.tensor.matmul(
                        pp[:, :T],
                        w_proj[ci][:, 128 * co:128 * (co + 1)],
                        unT[ci][:, :],
                        start=(ci == 0), stop=(ci == 5))
                osb = p_osb.tile([128, T], F32, tag="osb", name="osb")
                nc.vector.tensor_add(
                    out=osb[:], in0=pp[:, :T],
                    in1=biast[:, co:co + 1].broadcast_to([128, T]))
                nc.sync.dma_start(
                    d_out[128 * co:128 * (co + 1), r0:r0 + T], osb[:])
                if co == 5:
                    del state[("unT", g)]

            # Per-step schedule: A = qkv group, B = attention group,
            # C = normalize+proj group. Attention/bc cells are interleaved
            # between big-GEMM chunks so their LDWEIGHTS hide under long
            # matmuls and ACT/DVE latencies never stall the PE.
            STEP = [
                ("recip", "C", 0),
                ("qk", "A", 0), ("sc", "B", 0), ("qk", "A", 1),
                ("bc", "C", 0),
                ("qk", "A", 2), ("sc", "B", 1), ("qk", "A", 3),
                ("bc", "C", 1),
                ("qk", "A", 4), ("fin", "B", 0), ("qk", "A", 5),
                ("bc", "C", 2),
                ("qk", "A", 6), ("sc", "B", 2), ("qk", "A", 7),
                ("bc", "C", 3),
                ("qk", "A", 8), ("fin", "B", 1), ("qk", "A", 9),
                ("bc", "C", 4),
                ("qk", "A", 10), ("sc", "B", 3), ("qk", "A", 11),
                ("bc", "C", 5), ("xdma", "A", 1),
                ("v", "A", 0), ("fin", "B", 2), ("proj", "C", 0),
                ("sc", "B", 4),
                ("v", "A", 1), ("proj", "C", 1), ("fin", "B", 3),
                ("sc", "B", 5),
                ("v", "A", 2), ("proj", "C", 2), ("fin", "B", 4),
                ("v", "A", 3), ("proj", "C", 3), ("fin", "B", 5),
                ("proj", "C", 4), ("proj", "C", 5),
            ]
            FN = {"qk": qk_cotile, "v": v_tile, "sc": attn_sc,
                  "fin": attn_fin, "bc": bcmul, "proj": proj_co}
            for step in range(N_GROUPS + 2):
                A, B_, C_ = step, step - 1, step - 2
                for item in STEP:
                    op, grp, idx = item
                    g = {"A": A, "B": B_, "C": C_}[grp]
                    if op == "xdma":
                        t = A + 1
                        if 0 <= t < N_GROUPS and ("xv", t) not in state:
                            issue_x_dma(t)
                        continue
                    if op == "recip":
                        if 0 <= g < N_GROUPS:
                            recip(g)
                        continue
                    if op in ("qk", "v") and not (0 <= g < N_GROUPS):
                        continue
                    if op in ("sc", "fin") and not (0 <= g < N_GROUPS):
                        continue
                    if op in ("bc", "proj") and not (0 <= g < N_GROUPS):
                        continue
                    FN[op](g, idx)

    nc.compile()
    return nc


def make_in_maps(inputs):
    """Host-side prep: shard + transpose + cast. inputs keys as in
    reference.setup_inputs()."""
    import ml_dtypes
    bf16 = ml_dtypes.bfloat16

    x = np.asarray(inputs["x"], dtype=np.float32)
    B, N, Cc = x.shape
    W_qkv = np.ascontiguousarray(
        np.asarray(inputs["W_qkv"], dtype=np.float32).astype(bf16))
    W_proj = np.ascontiguousarray(
        np.asarray(inputs["W_proj"], dtype=np.float32).astype(bf16))
    biast = np.ascontiguousarray(
        np.asarray(inputs["b_proj"], dtype=np.float32)
        .reshape(6, 128).T.astype(bf16))
    consts = _consts()

    in_maps = []
    for i in range(NUM_CORES):
        xt = np.zeros((Cc, TOK + 2), dtype=bf16)
        xt[:, :TOK] = (
            x[i * B_CORE:(i + 1) * B_CORE].reshape(TOK, Cc).T.astype(bf16))
        m = {"xt": xt, "wqkv": W_qkv, "wproj": W_proj, "biast": biast}
        m.update(consts)
        in_maps.append(m)
    return in_maps


def kernel(x, W_qkv, W_proj, b_proj):
    from concourse.bass_utils import run_bass_kernel_spmd

    if "nc" not in _CACHE:
        _CACHE["nc"] = _build()
    nc = _CACHE["nc"]

    in_maps = make_in_maps(
        {"x": x, "W_qkv": W_qkv, "W_proj": W_proj, "b_proj": b_proj})
    res = run_bass_kernel_spmd(nc, in_maps, list(range(NUM_CORES)))
    B, N, Cc = np.asarray(x).shape
    out = np.empty((B, N, Cc), dtype=np.float32)
    for i in range(NUM_CORES):
        out[i * B_CORE:(i + 1) * B_CORE] = res.results[i]["out"].T.reshape(
            B_CORE, N, Cc)
    return out


# revision 29
# speedup vs baseline: 1.0026x; 1.0026x over previous
"""Trainium2 Bass kernel: fused attention block (QKV proj -> MHA -> out proj).

Reference (per batch item b, NUM_HEADS=12, Dh=64):
    qkv = x @ W_qkv; q,k,v per head
    attn = softmax(q @ k^T / 8) @ v
    out  = concat_heads(attn) @ W_proj + b_proj

Sharding: data-parallel over batch across 8 NeuronCores (128 batch items
per core), weights replicated. One SPMD Bass program, per-core inputs.

Per-core plan (128 batches, groups of G=8 batches = 392 tokens). All
matmul operands bf16 (fp32 accumulate in PSUM). x is pre-transposed +
cast on host to feature-major xT[768, 6272] per core, so no on-device
transposes. Software-pipelined one group deep: attention+proj of group
g-1 is emitted alongside QKV of group g so the PE never idles.

  B. q,k GEMM feature-major: psum[128co, T] = Wqkv_tile.T @ xT.
     q co-tiles stored naturally [128, T+2]; k co-tiles DVE-scattered
     into block-diagonal kbd[g%2]: per (j, b) a [128, 128] block with
     k_h(2j) rows 0:64 cols 0:49 and k_h(2j+1) rows 64:128 cols 64:113
     (zeros elsewhere, memset once).
  C. v GEMM token-major over 98-token tiles -> scratch; 4 aligned
     SBUF->SBUF DMAs per tile scatter into vbd[g%2]: per (j, b) a
     [128, 128] block with v_h(2j) rows 0:49 cols 0:64 and v_h(2j+1)
     rows 64:113 cols 64:128.
  D. attention per (head-pair j, batch): both heads in one matmul chain:
       sT2 = kbd_jb.T @ q_pair        [128, 50]  scores, heads stacked
       eT2 = exp(sT2/8)               ACT, [128, 400] per j
       r2  = onesbd.T @ eT2           [2, 400]   row sums per head
       po  = vbd_jb.T @ eT2           [128, 50]  unnorm out^T, both heads
     r rows gathered (ACT copy + SBUF-shift DMA) into rgrp[12, T];
     one batched reciprocal per group; per j: broadcast matmul
     (sel2 selector, f32r) -> psum bc[128, T]; unT[j] *= bc (DVE).
  E. proj GEMM token-major: psum[98tok, 384] = unT_slice.T @ Wproj,
     bias accumulated via ones-row matmul -> copy -> DMA out.
"""
import sys

sys.path.insert(0, "/opt/trn_rl_repo")

import numpy as np

NUM_CORES = 8
B_CORE = 128          # batch items per core
SEQ = 49              # tokens per batch item
C = 768               # channels
H = 12                # heads
G = 8                 # batch items per group
T = SEQ * G           # 392 tokens per group (even)
TP = T + 2            # padded q tile width
TOK = B_CORE * SEQ    # 6272 tokens per core
N_GROUPS = B_CORE // G
BDW = G * 128         # kbd/vbd block row width per j (padded 128/batch)

_CACHE = {}


def _consts():
    import ml_dtypes
    bf16 = ml_dtypes.bfloat16
    # sel2: broadcast head sums over Dh partitions (bf16 matmul);
    # contraction padded to 128 rows (sub-128-partition matmuls run slow)
    sel2 = np.zeros((128, 6 * 128), dtype=bf16)
    for j in range(6):
        sel2[2 * j, 128 * j:128 * j + 64] = 1.0
        sel2[2 * j + 1, 128 * j + 64:128 * (j + 1)] = 1.0
    # onesbd6: per head-pair j a [128, 32] stationary with ones at col 2j
    # (head-even keys, eT rows 0:49) and col 2j+1 (head-odd, rows 64:113);
    # all 6 r matmuls accumulate into one [32, 400] psum tile
    onesbd = np.zeros((128, 6 * 128), dtype=bf16)
    for j in range(6):
        onesbd[0:49, 128 * j + 2 * j] = 1.0
        onesbd[64:113, 128 * j + 2 * j + 1] = 1.0
    return {"sel2": sel2, "onesbd": onesbd}


def _build():
    import concourse.bacc as bacc
    import concourse.mybir as mybir
    import concourse.tile as tile

    F32 = mybir.dt.float32
    F32R = mybir.dt.float32r
    BF16 = mybir.dt.bfloat16
    EXP = mybir.ActivationFunctionType.Exp

    nc = bacc.Bacc("TRN2", target_bir_lowering=False)

    d_xt = nc.declare_dram_parameter("xt", [C, TOK + 2], BF16, isOutput=False)
    d_wqkv = nc.declare_dram_parameter("wqkv", [C, 3 * C], BF16, isOutput=False)
    d_wproj = nc.declare_dram_parameter("wproj", [C, C], BF16, isOutput=False)
    d_biast = nc.declare_dram_parameter("biast", [128, 6], BF16,
                                        isOutput=False)
    d_sel2 = nc.declare_dram_parameter("sel2", [128, 6 * 128], BF16,
                                       isOutput=False)
    d_onesbd = nc.declare_dram_parameter("onesbd", [128, 6 * 128], BF16,
                                         isOutput=False)
    d_out = nc.declare_dram_parameter("out", [C, TOK], F32, isOutput=True)

    # 98-token tiles within a group (4 per group, batch-pair aligned)
    tts = [(o, 98) for o in range(0, T, 98)]

    with tile.TileContext(nc) as tc, \
         nc.allow_low_precision(reason="bf16 operands, fp32 accumulate"):
        with tc.tile_pool(name="wres", bufs=1) as wres, \
             tc.tile_pool(name="xT", bufs=2) as p_xT, \
             tc.tile_pool(name="qk", bufs=2) as p_qk, \
             tc.tile_pool(name="vscr", bufs=2) as p_vscr, \
             tc.tile_pool(name="eT", bufs=2) as p_eT, \
             tc.tile_pool(name="rr", bufs=2) as p_rr, \
             tc.tile_pool(name="unT", bufs=3) as p_unT, \
             tc.tile_pool(name="osb", bufs=2) as p_osb, \
             tc.tile_pool(name="psA", bufs=2, space="PSUM") as psA, \
             tc.tile_pool(name="psB", bufs=2, space="PSUM") as psB, \
             tc.tile_pool(name="psS", bufs=1, space="PSUM") as psS, \
             tc.tile_pool(name="psO", bufs=2, space="PSUM") as psO, \
             tc.tile_pool(name="psR", bufs=1, space="PSUM") as psR:

            state = {}

            def issue_x_dma(g, split=False):
                xT = p_xT.tile([128, 6 * TP], BF16, tag="xT", name="xT")
                xv = xT[:].rearrange("p (ci t) -> p ci t", t=TP)
                src_ap = d_xt[:].rearrange("(ci p) t -> p ci t", ci=6)
                if split:
                    for ci in range(6):
                        nc.sync.dma_start(
                            xv[:, ci:ci + 1, :],
                            src_ap[:, ci:ci + 1, g * T:g * T + TP])
                else:
                    nc.sync.dma_start(xv, src_ap[:, :, g * T:g * T + TP])
                state[("xv", g)] = xv

            # groups 0+1's x first so the PE can start ASAP
            issue_x_dma(0, split=True)
            issue_x_dma(1, split=True)

            # ---- resident weights / constants ----
            w_qkv = []
            for ci in range(6):
                t = wres.tile([128, 3 * C], BF16, tag=f"wqkv{ci}")
                nc.sync.dma_start(t[:], d_wqkv[128 * ci:128 * (ci + 1), :])
                w_qkv.append(t)
            w_proj = []
            for ci in range(6):
                t = wres.tile([128, C], BF16, tag=f"wproj{ci}")
                nc.sync.dma_start(t[:], d_wproj[128 * ci:128 * (ci + 1), :])
                w_proj.append(t)
            sel2 = wres.tile([128, 6 * 128], BF16, tag="sel2")
            nc.sync.dma_start(sel2[:], d_sel2[:])
            onesbd = wres.tile([128, 6 * 128], BF16, tag="onesbd")
            nc.sync.dma_start(onesbd[:], d_onesbd[:])
            biast = wres.tile([128, 6], BF16, tag="biast")
            nc.sync.dma_start(biast[:], d_biast[:])
            # double-buffered block-diagonal k/v, zeroed once
            kbds, vbds = [], []
            for s in range(2):
                kb = wres.tile([128, 6 * BDW], BF16, tag=f"kbd{s}")
                nc.vector.memset(kb[:], 0.0)
                kbds.append(kb)
                vb = wres.tile([128, 6 * BDW], BF16, tag=f"vbd{s}")
                nc.vector.memset(vb[:], 0.0)
                vbds.append(vb)

            def qk_cotile(g, j):
                """One qkv co-tile: 6 MMs; q copy or k block-diag scatter."""
                kbd = kbds[g % 2]
                xv = state[("xv", g)]
                pq = psA.tile([128, TP], F32, tag="psA", name="psa")
                for ci in range(6):
                    nc.tensor.matmul(
                        pq[:, :TP],
                        w_qkv[ci][:, 128 * j:128 * (j + 1)],
                        xv[:, ci, :],
                        start=(ci == 0), stop=(ci == 5))
                if j < 6:
                    t = p_qk.tile([128, TP], BF16, tag=f"q{j}", name=f"q{j}")
                    nc.scalar.copy(t[:], pq[:])
                    state[("q", g, j)] = t
                else:
                    jj = j - 6
                    kv = kbd[:, jj * BDW:(jj + 1) * BDW].rearrange(
                        "p (b two t) -> p b two t", two=2, t=64)
                    nc.scalar.copy(
                        kv[0:64, :, 0, 0:49],
                        pq[0:64, :T].rearrange("p (b t) -> p b t", t=49))
                    nc.scalar.copy(
                        kv[64:128, :, 1, 0:49],
                        pq[64:128, :T].rearrange("p (b t) -> p b t", t=49))

            def v_tile(g, tti):
                """One 98-token v tile: 12 MMs + 2 copies + 4 scatter DMAs."""
                vbd = vbds[g % 2]
                xv = state[("xv", g)]
                v5 = vbd[:].rearrange("p (j b two c) -> p j b two c",
                                      b=G, two=2, c=64)
                to, tk = tts[tti]
                scr = p_vscr.tile([98, C], BF16, tag="vscr", name="vscr")
                for half in range(2):
                    pv = psB.tile([98, 384], F32, tag="psB", name="psb")
                    for ci in range(6):
                        nc.tensor.matmul(
                            pv[:, :],
                            xv[:, ci, to:to + tk],
                            w_qkv[ci][:, 1536 + 384 * half:
                                      1536 + 384 * (half + 1)],
                            start=(ci == 0), stop=(ci == 5))
                    nc.vector.tensor_copy(
                        scr[:, 384 * half:384 * (half + 1)], pv[:, :])
                sv = scr[:].rearrange("p (j two c) -> p j two c",
                                      two=2, c=64)
                for bl in range(2):
                    b = 2 * tti + bl
                    for two in range(2):
                        nc.sync.dma_start(
                            v5[64 * two:64 * two + 49, :, b, two, :],
                            sv[49 * bl:49 * bl + 49, :, two, :])

            def attn_sc(g, j):
                """Scores + exp for head-pair j of group g."""
                kbd = kbds[g % 2]
                if j == 0:
                    state[("unT", g)] = [
                        p_unT.tile([128, T], BF16, tag=f"unT{ci}",
                                   name=f"unT{ci}") for ci in range(6)]
                    state[("rall", g)] = psR.tile([128, 50 * G], F32,
                                                  tag="rall", name="rall")
                qj = state.pop(("q", g, j))
                ps = psS.tile([128, 50 * G], F32, tag="psS", name="ps")
                for b in range(G):
                    nc.tensor.matmul(
                        ps[:, 50 * b:50 * b + 50],
                        kbd[:, j * BDW + 128 * b:j * BDW + 128 * (b + 1)],
                        qj[:, 49 * b:49 * b + 50],
                        start=True, stop=True)
                eT = p_eT.tile([128, 50 * G], BF16, tag="eT", name="eT")
                nc.scalar.activation(eT[:], ps[:], EXP, scale=0.125)
                state[("eT", g, j)] = eT

            def attn_fin(g, j):
                """Row sums + po + unT extraction for head-pair j."""
                vbd = vbds[g % 2]
                eT = state.pop(("eT", g, j))
                unT = state[("unT", g)]
                rall = state[("rall", g)]
                nc.tensor.matmul(rall[:], onesbd[:, 128 * j:128 * (j + 1)],
                                 eT[:], start=(j == 0), stop=(j == 5))
                po = psO.tile([128, 50 * G], F32, tag="psO", name="po")
                for b in range(G):
                    nc.tensor.matmul(
                        po[:, 50 * b:50 * b + 50],
                        vbd[:, j * BDW + 128 * b:j * BDW + 128 * (b + 1)],
                        eT[:, 50 * b:50 * b + 50],
                        start=True, stop=True)
                nc.vector.tensor_copy(
                    unT[j][:, :].rearrange("p (b c) -> p b c", c=49),
                    po[:, :].rearrange("p (b c) -> p b c", c=50)[:, :, 0:49])
                if j == 5:
                    rall = state.pop(("rall", g))
                    rgrp = p_rr.tile([H, T], F32, tag="rgrp", name="rgrp")
                    nc.scalar.copy(
                        rgrp[:].rearrange("p (b c) -> p b c", c=49),
                        rall[0:H, :].rearrange(
                            "p (b c) -> p b c", c=50)[:, :, 0:49])
                    state[("rgrp", g)] = rgrp

            def recip(g):
                rgrp = state.pop(("rgrp", g))
                rr32 = p_rr.tile([H, T], F32, tag="rr32", name="rr32")
                nc.vector.reciprocal_approx_fast(rr32[:], rgrp[:])
                rr = p_rr.tile([128, T], BF16, tag="rr", name="rr")
                nc.vector.memset(rr[:], 0.0)
                nc.vector.tensor_copy(rr[0:H, :], rr32[:])
                state[("rr", g)] = rr

            def bcmul(g, j):
                """Broadcast 1/r over Dh partitions; normalize unT[j]."""
                rr = state[("rr", g)]
                unT = state[("unT", g)]
                pbc = psA.tile([128, TP], F32, tag="psA", name="psa")
                nc.tensor.matmul(pbc[:, :T], sel2[:, 128 * j:128 * (j + 1)],
                                 rr[:], start=True, stop=True)
                nc.vector.tensor_mul(out=unT[j][:, :], in0=unT[j][:, :],
                                     in1=pbc[:, :T])
                if j == 5:
                    del state[("rr", g)]

            def proj_co(g, co):
                """One out co-tile: 6 MMs + bias add + DMA out."""
                r0 = g * T
                unT = state[("unT", g)]
                pp = psA.tile([128, TP], F32, tag="psA", name="psa")
                for ci in range(6):
                    nc.tensor.matmul(
                        pp[:, :T],
                        w_proj[ci][:, 128 * co:128 * (co + 1)],
                        unT[ci][:, :],
                        start=(ci == 0), stop=(ci == 5))
                osb = p_osb.tile([128, T], F32, tag="osb", name="osb")
                nc.vector.tensor_add(
                    out=osb[:], in0=pp[:, :T],
                    in1=biast[:, co:co + 1].broadcast_to([128, T]))
                nc.sync.dma_start(
                    d_out[128 * co:128 * (co + 1), r0:r0 + T], osb[:])
                if co == 5:
                    del state[("unT", g)]

            # Per-step schedule: A = qkv group, B = attention group,
            # C = normalize+proj group. Attention/bc cells are interleaved
            # between big-GEMM chunks so their LDWEIGHTS hide under long
            # matmuls and ACT/DVE latencies never stall the PE.
            STEP = [
                ("recip", "C", 0),
                ("qk", "A", 0), ("sc", "B", 0), ("qk", "A", 1),
                ("bc", "C", 0),
                ("qk", "A", 2), ("sc", "B", 1), ("qk", "A", 3),
                ("bc", "C", 1),
                ("qk", "A", 4), ("fin", "B", 0), ("qk", "A", 5),
                ("bc", "C", 2),
                ("qk", "A", 6), ("sc", "B", 2), ("qk", "A", 7),
                ("bc", "C", 3),
                ("qk", "A", 8), ("fin", "B", 1), ("qk", "A", 9),
                ("bc", "C", 4),
                ("qk", "A", 10), ("sc", "B", 3), ("qk", "A", 11),
                ("bc", "C", 5), ("xdma", "A", 1),
                ("v", "A", 0), ("fin", "B", 2), ("proj", "C", 0),
                ("sc", "B", 4),
                ("v", "A", 1), ("proj", "C", 1), ("fin", "B", 3),
                ("sc", "B", 5),
                ("v", "A", 2), ("proj", "C", 2), ("fin", "B", 4),
                ("v", "A", 3), ("proj", "C", 3), ("fin", "B", 5),
                ("proj", "C", 4), ("proj", "C", 5),
            ]
            FN = {"qk": qk_cotile, "v": v_tile, "sc": attn_sc,
                  "fin": attn_fin, "bc": bcmul, "proj": proj_co}
            for step in range(N_GROUPS + 2):
                A, B_, C_ = step, step - 1, step - 2
                for item in STEP:
                    op, grp, idx = item
                    g = {"A": A, "B": B_, "C": C_}[grp]
                    if op == "xdma":
                        t = A + 1
                        if 0 <= t < N_GROUPS and ("xv", t) not in state:
                            issue_x_dma(t)
                        continue
                    if op == "recip":
                        if 0 <= g < N_GROUPS:
                            recip(g)
                        continue
                    if op in ("qk", "v") and not (0 <= g < N_GROUPS):
                        continue
                    if op in ("sc", "fin") and not (0 <= g < N_GROUPS):
                        continue
                    if op in ("bc", "proj") and not (0 <= g < N_GROUPS):
                        continue
                    FN[op](g, idx)

    nc.compile()
    return nc


def make_in_maps(inputs):
    """Host-side prep: shard + transpose + cast. inputs keys as in
    reference.setup_inputs()."""
    import ml_dtypes
    bf16 = ml_dtypes.bfloat16

    x = np.asarray(inputs["x"], dtype=np.float32)
    B, N, Cc = x.shape
    W_qkv = np.ascontiguousarray(
        np.asarray(inputs["W_qkv"], dtype=np.float32).astype(bf16))
    W_proj = np.ascontiguousarray(
        np.asarray(inputs["W_proj"], dtype=np.float32).astype(bf16))
    biast = np.ascontiguousarray(
        np.asarray(inputs["b_proj"], dtype=np.float32)
        .reshape(6, 128).T.astype(bf16))
    consts = _consts()

    in_maps = []
    for i in range(NUM_CORES):
        xt = np.zeros((Cc, TOK + 2), dtype=bf16)
        xt[:, :TOK] = (
            x[i * B_CORE:(i + 1) * B_CORE].reshape(TOK, Cc).T.astype(bf16))
        m = {"xt": xt, "wqkv": W_qkv, "wproj": W_proj, "biast": biast}
        m.update(consts)
        in_maps.append(m)
    return in_maps


def kernel(x, W_qkv, W_proj, b_proj):
    from concourse.bass_utils import run_bass_kernel_spmd

    if "nc" not in _CACHE:
        _CACHE["nc"] = _build()
    nc = _CACHE["nc"]

    in_maps = make_in_maps(
        {"x": x, "W_qkv": W_qkv, "W_proj": W_proj, "b_proj": b_proj})
    res = run_bass_kernel_spmd(nc, in_maps, list(range(NUM_CORES)))
    B, N, Cc = np.asarray(x).shape
    out = np.empty((B, N, Cc), dtype=np.float32)
    for i in range(NUM_CORES):
        out[i * B_CORE:(i + 1) * B_CORE] = res.results[i]["out"].T.reshape(
            B_CORE, N, Cc)
    return out


# revision 30
# speedup vs baseline: 1.0206x; 1.0179x over previous
"""Trainium2 Bass kernel: fused attention block (QKV proj -> MHA -> out proj).

Reference (per batch item b, NUM_HEADS=12, Dh=64):
    qkv = x @ W_qkv; q,k,v per head
    attn = softmax(q @ k^T / 8) @ v
    out  = concat_heads(attn) @ W_proj + b_proj

Sharding: data-parallel over batch across 8 NeuronCores (128 batch items
per core), weights replicated. One SPMD Bass program, per-core inputs.

Per-core plan (128 batches, groups of G=8 batches = 392 tokens). All
matmul operands bf16 (fp32 accumulate in PSUM). x is pre-transposed +
cast on host to feature-major xT[768, 6272] per core, so no on-device
transposes. Software-pipelined one group deep: attention+proj of group
g-1 is emitted alongside QKV of group g so the PE never idles.

  B. q,k GEMM feature-major: psum[128co, T] = Wqkv_tile.T @ xT.
     q co-tiles stored naturally [128, T+2]; k co-tiles DVE-scattered
     into block-diagonal kbd[g%2]: per (j, b) a [128, 128] block with
     k_h(2j) rows 0:64 cols 0:49 and k_h(2j+1) rows 64:128 cols 64:113
     (zeros elsewhere, memset once).
  C. v GEMM token-major over 98-token tiles -> scratch; 4 aligned
     SBUF->SBUF DMAs per tile scatter into vbd[g%2]: per (j, b) a
     [128, 128] block with v_h(2j) rows 0:49 cols 0:64 and v_h(2j+1)
     rows 64:113 cols 64:128.
  D. attention per (head-pair j, batch): both heads in one matmul chain:
       sT2 = kbd_jb.T @ q_pair        [128, 50]  scores, heads stacked
       eT2 = exp(sT2/8)               ACT, [128, 400] per j
       r2  = onesbd.T @ eT2           [2, 400]   row sums per head
       po  = vbd_jb.T @ eT2           [128, 50]  unnorm out^T, both heads
     r rows gathered (ACT copy + SBUF-shift DMA) into rgrp[12, T];
     one batched reciprocal per group; per j: broadcast matmul
     (sel2 selector, f32r) -> psum bc[128, T]; unT[j] *= bc (DVE).
  E. proj GEMM token-major: psum[98tok, 384] = unT_slice.T @ Wproj,
     bias accumulated via ones-row matmul -> copy -> DMA out.
"""
import sys

sys.path.insert(0, "/opt/trn_rl_repo")

import numpy as np

NUM_CORES = 8
B_CORE = 128          # batch items per core
SEQ = 49              # tokens per batch item
C = 768               # channels
H = 12                # heads
G = 8                 # batch items per group
T = SEQ * G           # 392 tokens per group (even)
TP = T + 2            # padded q tile width
TOK = B_CORE * SEQ    # 6272 tokens per core
N_GROUPS = B_CORE // G
BDW = G * 128         # kbd/vbd block row width per j (padded 128/batch)

_CACHE = {}


def _consts():
    import ml_dtypes
    bf16 = ml_dtypes.bfloat16
    # sel2: broadcast head sums over Dh partitions (bf16 matmul);
    # contraction padded to 128 rows (sub-128-partition matmuls run slow)
    sel2 = np.zeros((128, 6 * 128), dtype=bf16)
    for j in range(6):
        sel2[2 * j, 128 * j:128 * j + 64] = 1.0
        sel2[2 * j + 1, 128 * j + 64:128 * (j + 1)] = 1.0
    # onesbd6: per head-pair j a [128, 32] stationary with ones at col 2j
    # (head-even keys, eT rows 0:49) and col 2j+1 (head-odd, rows 64:113);
    # all 6 r matmuls accumulate into one [32, 400] psum tile
    onesbd = np.zeros((128, 6 * 128), dtype=bf16)
    for j in range(6):
        onesbd[0:49, 128 * j + 2 * j] = 1.0
        onesbd[64:113, 128 * j + 2 * j + 1] = 1.0
    return {"sel2": sel2, "onesbd": onesbd}


def _build():
    import concourse.bacc as bacc
    import concourse.mybir as mybir
    import concourse.tile as tile

    F32 = mybir.dt.float32
    F32R = mybir.dt.float32r
    BF16 = mybir.dt.bfloat16
    EXP = mybir.ActivationFunctionType.Exp

    nc = bacc.Bacc("TRN2", target_bir_lowering=False)

    d_xt = nc.declare_dram_parameter("xt", [C, TOK + 2], BF16, isOutput=False)
    d_wqkv = nc.declare_dram_parameter("wqkv", [C, 3 * C], BF16, isOutput=False)
    d_wproj = nc.declare_dram_parameter("wproj", [C, C], BF16, isOutput=False)
    d_biast = nc.declare_dram_parameter("biast", [128, 6], BF16,
                                        isOutput=False)
    d_sel2 = nc.declare_dram_parameter("sel2", [128, 6 * 128], BF16,
                                       isOutput=False)
    d_onesbd = nc.declare_dram_parameter("onesbd", [128, 6 * 128], BF16,
                                         isOutput=False)
    d_out = nc.declare_dram_parameter("out", [C, TOK], F32, isOutput=True)

    # 98-token tiles within a group (4 per group, batch-pair aligned)
    tts = [(o, 98) for o in range(0, T, 98)]

    with tile.TileContext(nc) as tc, \
         nc.allow_low_precision(reason="bf16 operands, fp32 accumulate"):
        with tc.tile_pool(name="wres", bufs=1) as wres, \
             tc.tile_pool(name="xT", bufs=2) as p_xT, \
             tc.tile_pool(name="qk", bufs=2) as p_qk, \
             tc.tile_pool(name="vscr", bufs=2) as p_vscr, \
             tc.tile_pool(name="eT", bufs=3) as p_eT, \
             tc.tile_pool(name="rr", bufs=2) as p_rr, \
             tc.tile_pool(name="unT", bufs=3) as p_unT, \
             tc.tile_pool(name="osb", bufs=2) as p_osb, \
             tc.tile_pool(name="psA", bufs=2, space="PSUM") as psA, \
             tc.tile_pool(name="psB", bufs=2, space="PSUM") as psB, \
             tc.tile_pool(name="psS", bufs=1, space="PSUM") as psS, \
             tc.tile_pool(name="psO", bufs=2, space="PSUM") as psO, \
             tc.tile_pool(name="psR", bufs=1, space="PSUM") as psR:

            state = {}

            def issue_x_dma(g, split=False):
                xT = p_xT.tile([128, 6 * TP], BF16, tag="xT", name="xT")
                xv = xT[:].rearrange("p (ci t) -> p ci t", t=TP)
                src_ap = d_xt[:].rearrange("(ci p) t -> p ci t", ci=6)
                if split:
                    for ci in range(6):
                        nc.sync.dma_start(
                            xv[:, ci:ci + 1, :],
                            src_ap[:, ci:ci + 1, g * T:g * T + TP])
                else:
                    nc.sync.dma_start(xv, src_ap[:, :, g * T:g * T + TP])
                state[("xv", g)] = xv

            # group 0's x first so the PE can start ASAP
            issue_x_dma(0, split=True)

            # ---- resident weights / constants ----
            w_qkv = []
            for ci in range(6):
                t = wres.tile([128, 3 * C], BF16, tag=f"wqkv{ci}")
                nc.sync.dma_start(t[:], d_wqkv[128 * ci:128 * (ci + 1), :])
                w_qkv.append(t)
            w_proj = []
            for ci in range(6):
                t = wres.tile([128, C], BF16, tag=f"wproj{ci}")
                nc.sync.dma_start(t[:], d_wproj[128 * ci:128 * (ci + 1), :])
                w_proj.append(t)
            sel2 = wres.tile([128, 6 * 128], BF16, tag="sel2")
            nc.sync.dma_start(sel2[:], d_sel2[:])
            onesbd = wres.tile([128, 6 * 128], BF16, tag="onesbd")
            nc.sync.dma_start(onesbd[:], d_onesbd[:])
            biast = wres.tile([128, 6], BF16, tag="biast")
            nc.sync.dma_start(biast[:], d_biast[:])
            issue_x_dma(1)
            # double-buffered block-diagonal k/v, zeroed once
            kbds, vbds = [], []
            for s in range(2):
                kb = wres.tile([128, 6 * BDW], BF16, tag=f"kbd{s}")
                nc.vector.memset(kb[:], 0.0)
                kbds.append(kb)
                vb = wres.tile([128, 6 * BDW], BF16, tag=f"vbd{s}")
                nc.vector.memset(vb[:], 0.0)
                vbds.append(vb)

            def qk_cotile(g, j):
                """One qkv co-tile: 6 MMs; q copy or k block-diag scatter."""
                kbd = kbds[g % 2]
                xv = state[("xv", g)]
                pq = psA.tile([128, TP], F32, tag="psA", name="psa")
                for ci in range(6):
                    nc.tensor.matmul(
                        pq[:, :TP],
                        w_qkv[ci][:, 128 * j:128 * (j + 1)],
                        xv[:, ci, :],
                        start=(ci == 0), stop=(ci == 5))
                if j < 6:
                    t = p_qk.tile([128, TP], BF16, tag=f"q{j}", name=f"q{j}")
                    nc.scalar.copy(t[:], pq[:])
                    state[("q", g, j)] = t
                else:
                    jj = j - 6
                    kv = kbd[:, jj * BDW:(jj + 1) * BDW].rearrange(
                        "p (b two t) -> p b two t", two=2, t=64)
                    nc.scalar.copy(
                        kv[0:64, :, 0, 0:49],
                        pq[0:64, :T].rearrange("p (b t) -> p b t", t=49))
                    nc.scalar.copy(
                        kv[64:128, :, 1, 0:49],
                        pq[64:128, :T].rearrange("p (b t) -> p b t", t=49))

            def v_tile(g, tti):
                """One 98-token v tile: 12 MMs + 2 copies + 4 scatter DMAs."""
                vbd = vbds[g % 2]
                xv = state[("xv", g)]
                v5 = vbd[:].rearrange("p (j b two c) -> p j b two c",
                                      b=G, two=2, c=64)
                to, tk = tts[tti]
                scr = p_vscr.tile([98, C], BF16, tag="vscr", name="vscr")
                for half in range(2):
                    pv = psB.tile([98, 384], F32, tag="psB", name="psb")
                    for ci in range(6):
                        nc.tensor.matmul(
                            pv[:, :],
                            xv[:, ci, to:to + tk],
                            w_qkv[ci][:, 1536 + 384 * half:
                                      1536 + 384 * (half + 1)],
                            start=(ci == 0), stop=(ci == 5))
                    nc.vector.tensor_copy(
                        scr[:, 384 * half:384 * (half + 1)], pv[:, :])
                sv = scr[:].rearrange("p (j two c) -> p j two c",
                                      two=2, c=64)
                for bl in range(2):
                    b = 2 * tti + bl
                    for two in range(2):
                        nc.sync.dma_start(
                            v5[64 * two:64 * two + 49, :, b, two, :],
                            sv[49 * bl:49 * bl + 49, :, two, :])

            def attn_sc(g, j):
                """Scores + exp for head-pair j of group g."""
                kbd = kbds[g % 2]
                if j == 0:
                    state[("unT", g)] = [
                        p_unT.tile([128, T], BF16, tag=f"unT{ci}",
                                   name=f"unT{ci}") for ci in range(6)]
                    state[("rall", g)] = psR.tile([128, 50 * G], F32,
                                                  tag="rall", name="rall")
                qj = state.pop(("q", g, j))
                ps = psS.tile([128, 50 * G], F32, tag="psS", name="ps")
                for b in range(G):
                    nc.tensor.matmul(
                        ps[:, 50 * b:50 * b + 50],
                        kbd[:, j * BDW + 128 * b:j * BDW + 128 * (b + 1)],
                        qj[:, 49 * b:49 * b + 50],
                        start=True, stop=True)
                eT = p_eT.tile([128, 50 * G], BF16, tag="eT", name="eT")
                nc.scalar.activation(eT[:], ps[:], EXP, scale=0.125)
                state[("eT", g, j)] = eT

            def attn_fin(g, j):
                """Row sums + po + unT extraction for head-pair j."""
                vbd = vbds[g % 2]
                eT = state.pop(("eT", g, j))
                unT = state[("unT", g)]
                rall = state[("rall", g)]
                nc.tensor.matmul(rall[:], onesbd[:, 128 * j:128 * (j + 1)],
                                 eT[:], start=(j == 0), stop=(j == 5))
                po = psO.tile([128, 50 * G], F32, tag="psO", name="po")
                for b in range(G):
                    nc.tensor.matmul(
                        po[:, 50 * b:50 * b + 50],
                        vbd[:, j * BDW + 128 * b:j * BDW + 128 * (b + 1)],
                        eT[:, 50 * b:50 * b + 50],
                        start=True, stop=True)
                nc.vector.tensor_copy(
                    unT[j][:, :].rearrange("p (b c) -> p b c", c=49),
                    po[:, :].rearrange("p (b c) -> p b c", c=50)[:, :, 0:49])
                if j == 5:
                    rall = state.pop(("rall", g))
                    rgrp = p_rr.tile([H, T], F32, tag="rgrp", name="rgrp")
                    nc.scalar.copy(
                        rgrp[:].rearrange("p (b c) -> p b c", c=49),
                        rall[0:H, :].rearrange(
                            "p (b c) -> p b c", c=50)[:, :, 0:49])
                    state[("rgrp", g)] = rgrp

            def recip(g):
                rgrp = state.pop(("rgrp", g))
                rr32 = p_rr.tile([H, T], F32, tag="rr32", name="rr32")
                nc.vector.reciprocal_approx_fast(rr32[:], rgrp[:])
                rr = p_rr.tile([128, T], BF16, tag="rr", name="rr")
                if g < 2:
                    nc.vector.memset(rr[:], 0.0)
                nc.vector.tensor_copy(rr[0:H, :], rr32[:])
                state[("rr", g)] = rr

            def bcmul(g, j):
                """Broadcast 1/r over Dh partitions; normalize unT[j]."""
                rr = state[("rr", g)]
                unT = state[("unT", g)]
                pbc = psA.tile([128, TP], F32, tag="psA", name="psa")
                nc.tensor.matmul(pbc[:, :T], sel2[:, 128 * j:128 * (j + 1)],
                                 rr[:], start=True, stop=True)
                nc.vector.tensor_mul(out=unT[j][:, :], in0=unT[j][:, :],
                                     in1=pbc[:, :T])
                if j == 5:
                    del state[("rr", g)]

            def proj_co(g, co):
                """One out co-tile: 6 MMs + bias add + DMA out."""
                r0 = g * T
                unT = state[("unT", g)]
                pp = psA.tile([128, TP], F32, tag="psA", name="psa")
                for ci in range(6):
                    nc.tensor.matmul(
                        pp[:, :T],
                        w_proj[ci][:, 128 * co:128 * (co + 1)],
                        unT[ci][:, :],
                        start=(ci == 0), stop=(ci == 5))
                osb = p_osb.tile([128, T], F32, tag="osb", name="osb")
                nc.vector.tensor_add(
                    out=osb[:], in0=pp[:, :T],
                    in1=biast[:, co:co + 1].broadcast_to([128, T]))
                nc.sync.dma_start(
                    d_out[128 * co:128 * (co + 1), r0:r0 + T], osb[:])
                if co == 5:
                    del state[("unT", g)]

            # Per-step schedule: A = qkv group, B = attention group,
            # C = normalize+proj group. Attention/bc cells are interleaved
            # between big-GEMM chunks so their LDWEIGHTS hide under long
            # matmuls and ACT/DVE latencies never stall the PE.
            STEP = [
                ("recip", "C", 0),
                ("qk", "A", 0), ("sc", "B", 0), ("qk", "A", 1),
                ("bc", "C", 0),
                ("qk", "A", 2), ("sc", "B", 1), ("qk", "A", 3),
                ("bc", "C", 1),
                ("qk", "A", 4), ("fin", "B", 0), ("qk", "A", 5),
                ("bc", "C", 2),
                ("qk", "A", 6), ("sc", "B", 2), ("qk", "A", 7),
                ("bc", "C", 3),
                ("qk", "A", 8), ("fin", "B", 1), ("qk", "A", 9),
                ("bc", "C", 4),
                ("qk", "A", 10), ("sc", "B", 3), ("qk", "A", 11),
                ("bc", "C", 5), ("xdma", "A", 1),
                ("v", "A", 0), ("fin", "B", 2), ("proj", "C", 0),
                ("sc", "B", 4),
                ("v", "A", 1), ("proj", "C", 1), ("fin", "B", 3),
                ("sc", "B", 5),
                ("v", "A", 2), ("proj", "C", 2), ("fin", "B", 4),
                ("v", "A", 3), ("proj", "C", 3), ("fin", "B", 5),
                ("proj", "C", 4), ("proj", "C", 5),
            ]
            FN = {"qk": qk_cotile, "v": v_tile, "sc": attn_sc,
                  "fin": attn_fin, "bc": bcmul, "proj": proj_co}
            for step in range(N_GROUPS + 2):
                A, B_, C_ = step, step - 1, step - 2
                for item in STEP:
                    op, grp, idx = item
                    g = {"A": A, "B": B_, "C": C_}[grp]
                    if op == "xdma":
                        t = A + 1
                        if 0 <= t < N_GROUPS and ("xv", t) not in state:
                            issue_x_dma(t)
                        continue
                    if op == "recip":
                        if 0 <= g < N_GROUPS:
                            recip(g)
                        continue
                    if op in ("qk", "v") and not (0 <= g < N_GROUPS):
                        continue
                    if op in ("sc", "fin") and not (0 <= g < N_GROUPS):
                        continue
                    if op in ("bc", "proj") and not (0 <= g < N_GROUPS):
                        continue
                    FN[op](g, idx)

    nc.compile()
    return nc


def make_in_maps(inputs):
    """Host-side prep: shard + transpose + cast. inputs keys as in
    reference.setup_inputs()."""
    import ml_dtypes
    bf16 = ml_dtypes.bfloat16

    x = np.asarray(inputs["x"], dtype=np.float32)
    B, N, Cc = x.shape
    W_qkv = np.ascontiguousarray(
        np.asarray(inputs["W_qkv"], dtype=np.float32).astype(bf16))
    W_proj = np.ascontiguousarray(
        np.asarray(inputs["W_proj"], dtype=np.float32).astype(bf16))
    biast = np.ascontiguousarray(
        np.asarray(inputs["b_proj"], dtype=np.float32)
        .reshape(6, 128).T.astype(bf16))
    consts = _consts()

    in_maps = []
    for i in range(NUM_CORES):
        xt = np.zeros((Cc, TOK + 2), dtype=bf16)
        xt[:, :TOK] = (
            x[i * B_CORE:(i + 1) * B_CORE].reshape(TOK, Cc).T.astype(bf16))
        m = {"xt": xt, "wqkv": W_qkv, "wproj": W_proj, "biast": biast}
        m.update(consts)
        in_maps.append(m)
    return in_maps


def kernel(x, W_qkv, W_proj, b_proj):
    from concourse.bass_utils import run_bass_kernel_spmd

    if "nc" not in _CACHE:
        _CACHE["nc"] = _build()
    nc = _CACHE["nc"]

    in_maps = make_in_maps(
        {"x": x, "W_qkv": W_qkv, "W_proj": W_proj, "b_proj": b_proj})
    res = run_bass_kernel_spmd(nc, in_maps, list(range(NUM_CORES)))
    B, N, Cc = np.asarray(x).shape
    out = np.empty((B, N, Cc), dtype=np.float32)
    for i in range(NUM_CORES):
        out[i * B_CORE:(i + 1) * B_CORE] = res.results[i]["out"].T.reshape(
            B_CORE, N, Cc)
    return out
